# revision 1
# baseline (speedup 1.0000x reference)
"""GCN (2-layer, PyG GCNConv semantics) on 8 Trainium2 NeuronCores.

Strategy
--------
Per-edge random gather/scatter primitives on TRN2 run at ~28-36ns/element
(SWDGE indirect descriptors / GpSimd ucode), which is 50-100x too slow for
16M edges. So all device work is DENSE: the host builds (as its
sharding/layout step) a dst-sorted, degree-padded edge grid per core, and
each NeuronCore does pure dense float math:

  grid[v_local, slot] holds x[src] (resp. y1[src]) and deg[src]+1 for the
  incoming edges of node v_local; segment-sum == row-sum over PAD slots.

Layer 1:  y1[v]   = dinv[v] * (sum_slots dinv_src*x_src + dinv[v]*x[v])
          (gcn_conv(x,W1,b1) == y1 outer W1 + b1 since C_in == 1)
Layer 2:  z_c[v]  = dinv[v] * (sum_slots dinv_src*relu(W1c*y1_src+b1c) + self)
          out     = z @ W2 + b2

Node ranges are sharded 8 ways (62500 nodes/core, edge counts balance to
~0.1%), so no collectives are needed; y1 is assembled on host between the
two NEFF launches (the only cross-layer dependency).

Pad slots carry (x=0, deg=1) so they contribute 0 to layer-1 sums; for
layer 2 a dense correction term removes the (PAD - cnt_v)*relu(b1c)
contribution of pad slots, keeping the kernel exact for any b1.
"""
import math
import sys

sys.path.insert(0, "/opt/trn_rl_repo")

import numpy as np

N_NODES = 500_000
N_EDGES = 16_000_000
N_CORES = 8
NPC = N_NODES // N_CORES        # nodes per core
NROWPP = 492                    # grid rows per partition (128*492 = 62976 >= NPC)
NROW = 128 * NROWPP
NCHUNK = 12
CROWS = NROWPP // NCHUNK        # rows per partition per chunk

_NEFF_CACHE: dict = {}


def _dinv_tiles(nc, pool, deg_u16_ap, shape, tag):
    """cast u16 deg -> f32, return (degf_tile, dinv_tile) aps."""
    from concourse import mybir

    degf = pool.tile(shape, mybir.dt.float32, tag=tag + "df")
    sq = pool.tile(shape, mybir.dt.float32, tag=tag + "sq")
    dnv = pool.tile(shape, mybir.dt.float32, tag=tag + "dv")
    nc.vector.tensor_copy(out=degf[:], in_=deg_u16_ap)
    nc.scalar.sqrt(out=sq[:], in_=degf[:])
    nc.vector.reciprocal(out=dnv[:], in_=sq[:])
    return degf, dnv


def _build_neff_a(PAD):
    from concourse import bacc, mybir, tile

    nc = bacc.Bacc("TRN2", target_bir_lowering=False, debug=False,
                   num_devices=N_CORES)
    f32, u16 = mybir.dt.float32, mybir.dt.uint16
    gx = nc.dram_tensor("gx", [128, NROWPP * PAD], f32, kind="ExternalInput")
    gd = nc.dram_tensor("gd", [128, NROWPP * PAD], u16, kind="ExternalInput")
    xo = nc.dram_tensor("xo", [128, NROWPP], f32, kind="ExternalInput")
    do = nc.dram_tensor("do_", [128, NROWPP], u16, kind="ExternalInput")
    y1 = nc.dram_tensor("y1", [128, NROWPP], f32, kind="ExternalOutput")

    with tile.TileContext(nc) as tc:
        with tc.tile_pool(name="p", bufs=2) as pool, \
             tc.tile_pool(name="q", bufs=1) as psm, \
             tc.tile_pool(name="s", bufs=1) as spool:
            seg = spool.tile([128, NROWPP], f32)
            for k in range(NCHUNK):
                sl = slice(k * CROWS * PAD, (k + 1) * CROWS * PAD)
                gxt = pool.tile([128, CROWS * PAD], f32, tag="gx")
                gdt = pool.tile([128, CROWS * PAD], u16, tag="gd")
                nc.sync.dma_start(out=gxt[:], in_=gx.ap()[:, sl])
                nc.sync.dma_start(out=gdt[:], in_=gd.ap()[:, sl])
                _, dnv = _dinv_tiles(nc, pool, gdt[:], [128, CROWS * PAD], "c")
                nc.vector.tensor_tensor(out=gxt[:], in0=gxt[:], in1=dnv[:],
                                        op=mybir.AluOpType.mult)
                nc.vector.tensor_reduce(
                    out=seg[:, k * CROWS:(k + 1) * CROWS],
                    in_=gxt[:].rearrange("p (c s) -> p c s", s=PAD),
                    axis=mybir.AxisListType.X, op=mybir.AluOpType.add)
            # finalize: y1 = dinv_own * (seg + dinv_own * x_own)
            xot = psm.tile([128, NROWPP], f32, tag="xo")
            dot = psm.tile([128, NROWPP], u16, tag="do")
            nc.sync.dma_start(out=xot[:], in_=xo.ap())
            nc.sync.dma_start(out=dot[:], in_=do.ap())
            _, dno = _dinv_tiles(nc, psm, dot[:], [128, NROWPP], "o")
            nc.vector.tensor_tensor(out=xot[:], in0=xot[:], in1=dno[:],
                                    op=mybir.AluOpType.mult)
            nc.vector.tensor_add(out=seg[:], in0=seg[:], in1=xot[:])
            nc.vector.tensor_tensor(out=seg[:], in0=seg[:], in1=dno[:],
                                    op=mybir.AluOpType.mult)
            nc.sync.dma_start(out=y1.ap(), in_=seg[:])
    nc.compile()
    return nc


def _build_neff_b(PAD):
    from concourse import bacc, mybir, tile

    nc = bacc.Bacc("TRN2", target_bir_lowering=False, debug=False,
                   num_devices=N_CORES)
    f32, u16 = mybir.dt.float32, mybir.dt.uint16
    Relu = mybir.ActivationFunctionType.Relu
    Ident = mybir.ActivationFunctionType.Identity
    Copy = mybir.ActivationFunctionType.Copy
    mult, add, sub = (mybir.AluOpType.mult, mybir.AluOpType.add,
                      mybir.AluOpType.subtract)

    gy = nc.dram_tensor("gy", [128, NROWPP * PAD], f32, kind="ExternalInput")
    gd = nc.dram_tensor("gd", [128, NROWPP * PAD], u16, kind="ExternalInput")
    y1o = nc.dram_tensor("y1o", [128, NROWPP], f32, kind="ExternalInput")
    do = nc.dram_tensor("do_", [128, NROWPP], u16, kind="ExternalInput")
    w1r = nc.dram_tensor("w1r", [128, 4], f32, kind="ExternalInput")
    b1r = nc.dram_tensor("b1r", [128, 4], f32, kind="ExternalInput")
    w2r = nc.dram_tensor("w2r", [128, 16], f32, kind="ExternalInput")
    b2r = nc.dram_tensor("b2r", [128, 4], f32, kind="ExternalInput")
    out = nc.dram_tensor("out", [128, NROWPP * 4], f32, kind="ExternalOutput")

    with tile.TileContext(nc) as tc:
        with tc.tile_pool(name="p", bufs=2) as pool, \
             tc.tile_pool(name="q", bufs=1) as psm, \
             tc.tile_pool(name="s", bufs=1) as spool:
            S = spool.tile([128, 4 * NROWPP], f32)          # per-channel sums
            w1t = spool.tile([128, 4], f32)
            b1t = spool.tile([128, 4], f32)
            rb1t = spool.tile([128, 4], f32)
            w2t = spool.tile([128, 16], f32)
            b2t = spool.tile([128, 4], f32)
            nc.sync.dma_start(out=w1t[:], in_=w1r.ap())
            nc.sync.dma_start(out=b1t[:], in_=b1r.ap())
            nc.sync.dma_start(out=w2t[:], in_=w2r.ap())
            nc.sync.dma_start(out=b2t[:], in_=b2r.ap())
            nc.scalar.activation(out=rb1t[:], in_=b1t[:], func=Relu)

            for k in range(NCHUNK):
                sl = slice(k * CROWS * PAD, (k + 1) * CROWS * PAD)
                gyt = pool.tile([128, CROWS * PAD], f32, tag="gy")
                gdt = pool.tile([128, CROWS * PAD], u16, tag="gd")
                nc.sync.dma_start(out=gyt[:], in_=gy.ap()[:, sl])
                nc.sync.dma_start(out=gdt[:], in_=gd.ap()[:, sl])
                _, dnv = _dinv_tiles(nc, pool, gdt[:], [128, CROWS * PAD], "c")
                for c in range(4):
                    t = pool.tile([128, CROWS * PAD], f32, tag="tch")
                    nc.scalar.activation(out=t[:], in_=gyt[:], func=Relu,
                                         bias=b1t[:, c:c + 1],
                                         scale=w1t[:, c:c + 1])
                    nc.vector.tensor_tensor(out=t[:], in0=t[:], in1=dnv[:],
                                            op=mult)
                    nc.vector.tensor_reduce(
                        out=S[:, c * NROWPP + k * CROWS:
                              c * NROWPP + (k + 1) * CROWS],
                        in_=t[:].rearrange("p (c s) -> p c s", s=PAD),
                        axis=mybir.AxisListType.X, op=add)

            # finalize
            y1t = psm.tile([128, NROWPP], f32, tag="y1o")
            dot = psm.tile([128, NROWPP], u16, tag="do")
            nc.sync.dma_start(out=y1t[:], in_=y1o.ap())
            nc.sync.dma_start(out=dot[:], in_=do.ap())
            degf, dno = _dinv_tiles(nc, psm, dot[:], [128, NROWPP], "o")
            ot = spool.tile([128, NROWPP * 4], f32)
            o3 = ot[:].rearrange("p (r j) -> p r j", j=4)
            tmp = psm.tile([128, NROWPP], f32, tag="tmp")
            for c in range(4):
                Sc = S[:, c * NROWPP:(c + 1) * NROWPP]
                # pad-slot correction: (degf - (PAD+1)) * rb1c  ==
                # -(PAD - cnt_v) * relu(b1c);  add it to Sc.
                nc.vector.scalar_tensor_tensor(
                    out=tmp[:], in0=degf[:], scalar=float(PAD + 1), in1=degf[:],
                    op0=sub, op1=mybir.AluOpType.bypass)
                nc.vector.scalar_tensor_tensor(
                    out=tmp[:], in0=tmp[:], scalar=rb1t[:, c:c + 1], in1=Sc,
                    op0=mult, op1=add)
                # self message: dinv_v * relu(W1c*y1_v + b1c)
                nc.scalar.activation(out=Sc, in_=y1t[:], func=Relu,
                                     bias=b1t[:, c:c + 1],
                                     scale=w1t[:, c:c + 1])
                nc.vector.tensor_tensor(out=Sc, in0=Sc, in1=dno[:], op=mult)
                nc.vector.tensor_add(out=Sc, in0=Sc, in1=tmp[:])
                # z_c = dinv_v * (...)
                nc.vector.tensor_tensor(out=Sc, in0=Sc, in1=dno[:], op=mult)
            for j in range(4):
                acc = psm.tile([128, NROWPP], f32, tag="acc")
                nc.scalar.activation(out=acc[:],
                                     in_=S[:, 0 * NROWPP:1 * NROWPP],
                                     func=Copy, scale=w2t[:, j:j + 1])
                for c in range(1, 4):
                    nc.vector.scalar_tensor_tensor(
                        out=acc[:], in0=S[:, c * NROWPP:(c + 1) * NROWPP],
                        scalar=w2t[:, c * 4 + j:c * 4 + j + 1], in1=acc[:],
                        op0=mult, op1=add)
                nc.scalar.activation(out=o3[:, :, j], in_=acc[:], func=Ident,
                                     bias=b2t[:, j:j + 1])
            nc.sync.dma_start(out=out.ap(), in_=ot[:])
    nc.compile()
    return nc


def _get_neffs(PAD):
    if PAD not in _NEFF_CACHE:
        _NEFF_CACHE[PAD] = (_build_neff_a(PAD), _build_neff_b(PAD))
    return _NEFF_CACHE[PAD]


def kernel(x, edge_index, W1, b1, W2, b2):
    from concourse import bass_utils

    x = np.asarray(x, dtype=np.float32)
    W1 = np.asarray(W1, dtype=np.float32)
    b1 = np.asarray(b1, dtype=np.float32)
    W2 = np.asarray(W2, dtype=np.float32)
    b2 = np.asarray(b2, dtype=np.float32)
    ei = np.asarray(edge_index)
    assert x.shape == (N_NODES, 1) and ei.shape == (2, N_EDGES)
    xf = np.ascontiguousarray(x.reshape(-1))
    src = ei[0].astype(np.int64)
    dst = ei[1].astype(np.int64)

    # ---- host layout (index work only) ----
    key = (dst << 19) | src                 # N_NODES < 2**19
    key.sort(kind="stable")
    sdst = key >> 19
    ssrc = (key & 0x7FFFF).astype(np.int64)
    deg = np.bincount(dst, minlength=N_NODES)
    maxdeg = int(deg.max())
    PAD = max(64, 16 * math.ceil((maxdeg + 1) / 16))
    degp1 = (deg + 1).astype(np.uint16)
    assert maxdeg + 1 < 65536
    ptr = np.zeros(N_NODES + 1, np.int64)
    np.cumsum(deg, out=ptr[1:])
    rank = np.arange(N_EDGES, dtype=np.int64) - ptr[sdst]
    corei = sdst // NPC
    flat = (sdst - corei * NPC) * PAD + rank

    GX = np.zeros((N_CORES, NROW * PAD), np.float32)
    GD = np.ones((N_CORES, NROW * PAD), np.uint16)
    GX[corei, flat] = xf[ssrc]
    GD[corei, flat] = degp1[ssrc]
    XO = np.zeros((N_CORES, NROW), np.float32)
    DO = np.ones((N_CORES, NROW), np.uint16)
    XO[:, :NPC] = xf.reshape(N_CORES, NPC)
    DO[:, :NPC] = degp1.reshape(N_CORES, NPC)

    nc_a, nc_b = _get_neffs(PAD)
    in_a = [{
        "gx": GX[c].reshape(128, NROWPP * PAD),
        "gd": GD[c].reshape(128, NROWPP * PAD),
        "xo": XO[c].reshape(128, NROWPP),
        "do_": DO[c].reshape(128, NROWPP),
    } for c in range(N_CORES)]
    res_a = bass_utils.run_bass_kernel_spmd(nc_a, in_a,
                                            core_ids=list(range(N_CORES)))
    y1 = np.concatenate(
        [res_a.results[c]["y1"].reshape(-1)[:NPC] for c in range(N_CORES)])

    GY = GX  # reuse buffer: same placement, new values
    GY[corei, flat] = y1[ssrc]
    Y1O = np.zeros((N_CORES, NROW), np.float32)
    Y1O[:, :NPC] = y1.reshape(N_CORES, NPC)
    w1r = np.tile(W1.reshape(1, 4), (128, 1)).astype(np.float32)
    b1r = np.tile(b1.reshape(1, 4), (128, 1)).astype(np.float32)
    w2r = np.tile(W2.reshape(1, 16), (128, 1)).astype(np.float32)
    b2r = np.tile(b2.reshape(1, 4), (128, 1)).astype(np.float32)
    in_b = [{
        "gy": GY[c].reshape(128, NROWPP * PAD),
        "gd": GD[c].reshape(128, NROWPP * PAD),
        "y1o": Y1O[c].reshape(128, NROWPP),
        "do_": DO[c].reshape(128, NROWPP),
        "w1r": w1r, "b1r": b1r, "w2r": w2r, "b2r": b2r,
    } for c in range(N_CORES)]
    res_b = bass_utils.run_bass_kernel_spmd(nc_b, in_b,
                                            core_ids=list(range(N_CORES)))
    out = np.concatenate(
        [res_b.results[c]["out"].reshape(-1, 4)[:NPC] for c in range(N_CORES)])
    return np.ascontiguousarray(out, dtype=np.float32)



# revision 2
# speedup vs baseline: 4.5404x; 4.5404x over previous
"""GCN (2-layer, PyG GCNConv semantics) on 8 Trainium2 NeuronCores.

Strategy
--------
All device work is DENSE row-sums over host-built, dst-sorted edge grids
(per-edge gather/scatter on TRN2 is far too slow). Versus the naive
grid approach, three things keep the device near the HBM roofline:

1. No per-slot normalization math. A tiny NEFF0 computes per-NODE
   u = dinv*x and dinv (dinv = 1/sqrt(deg+1)); the host gathers the
   already-normalized per-node values into the edge grids, so the big
   NEFFs only do tensor_reduce row-sums (the naive per-slot
   cast/sqrt/reciprocal/mult costs ~400us of DVE time per layer).
2. fp16 grids: halves HBM traffic and doubles DVE reduce throughput.
   (fp16 values are produced ON DEVICE; the host only moves bytes.)
3. Degree-sorted rows with per-chunk padding: nodes are sorted by
   degree (desc) inside each core, rows take 128 consecutive ranks, and
   each chunk of rows is padded to its own max degree -> padding
   inflation ~1.14x instead of maxdeg/meandeg ~2.5x.

Math (A = D^-1/2 (Adj+I) D^-1/2, deg counts in-edges at dst +1):
  y1[v]   = dinv[v]*(sum_{e->v} u[src] + u[v]),  u = dinv*x
  M[v,c]  = dinv[v]*relu(W1[0,c]*y1[v] + b1[c])
  z[v,c]  = dinv[v]*(sum_{e->v} M[src,c] + M[v,c])
  out     = z @ W2 + b2

NEFF0: per-node u, dinv.  NEFF1: layer-1 row-sums + M.  NEFF2: 4-channel
row-sums + W2 combine. Host work between launches is pure index
work (sort/gather/scatter/pad of device-produced bytes).
"""
import math
import sys

sys.path.insert(0, "/opt/trn_rl_repo")

import numpy as np

N_NODES = 500_000
N_EDGES = 16_000_000
N_CORES = 8
NPC = N_NODES // N_CORES        # 62500 nodes per core
NROWPP = 496                    # rows per partition (128*496 = 63488 >= NPC)
NROWTOT = 128 * NROWPP
NCHUNK = 16
CROWS = NROWPP // NCHUNK        # 31 rows per partition per chunk

_NEFF_CACHE: dict = {}


def _build_neff0():
    """Per-node: dinv = 1/sqrt(deg+1); u = x*dinv (fp16 out)."""
    from concourse import bacc, mybir, tile

    nc = bacc.Bacc("TRN2", target_bir_lowering=False, debug=False,
                   num_devices=N_CORES)
    f32, f16, u16 = mybir.dt.float32, mybir.dt.float16, mybir.dt.uint16
    xo = nc.dram_tensor("xo", [128, NROWPP], f32, kind="ExternalInput")
    dg = nc.dram_tensor("dg", [128, NROWPP], u16, kind="ExternalInput")
    uo = nc.dram_tensor("uo", [128, NROWPP], f16, kind="ExternalOutput")
    dv = nc.dram_tensor("dv", [128, NROWPP], f32, kind="ExternalOutput")

    with tile.TileContext(nc) as tc:
        with tc.tile_pool(name="p", bufs=1) as pool:
            sh = [128, NROWPP]
            xt = pool.tile(sh, f32, tag="x")
            dt_ = pool.tile(sh, u16, tag="d")
            nc.sync.dma_start(out=xt[:], in_=xo.ap())
            nc.sync.dma_start(out=dt_[:], in_=dg.ap())
            df = pool.tile(sh, f32, tag="df")
            nc.vector.tensor_copy(out=df[:], in_=dt_[:])
            sq = pool.tile(sh, f32, tag="sq")
            nc.scalar.sqrt(out=sq[:], in_=df[:])
            dvt = pool.tile(sh, f32, tag="dv")
            nc.vector.reciprocal(out=dvt[:], in_=sq[:])
            u32 = pool.tile(sh, f32, tag="u32")
            nc.vector.tensor_tensor(out=u32[:], in0=xt[:], in1=dvt[:],
                                    op=mybir.AluOpType.mult)
            ut = pool.tile(sh, f16, tag="u")
            nc.vector.tensor_copy(out=ut[:], in_=u32[:])
            nc.sync.dma_start(out=uo.ap(), in_=ut[:])
            nc.sync.dma_start(out=dv.ap(), in_=dvt[:])
    nc.compile()
    return nc


def _build_neff1(pads):
    """Layer 1: seg[v] = rowsum(u[src]); y1 = dinv*(seg+u);
    M[v,c] = dinv*relu(W1c*y1+b1c) -> fp16 planes [128, 4*NROWPP]."""
    from concourse import bacc, mybir, tile

    nc = bacc.Bacc("TRN2", target_bir_lowering=False, debug=False,
                   num_devices=N_CORES)
    f32, f16 = mybir.dt.float32, mybir.dt.float16
    mult, add = mybir.AluOpType.mult, mybir.AluOpType.add
    Relu = mybir.ActivationFunctionType.Relu
    totw = sum(CROWS * p for p in pads)
    padmax = max(pads)

    gu = nc.dram_tensor("gu", [128, totw], f16, kind="ExternalInput")
    uo = nc.dram_tensor("uo", [128, NROWPP], f16, kind="ExternalInput")
    dv = nc.dram_tensor("dv", [128, NROWPP], f32, kind="ExternalInput")
    w1r = nc.dram_tensor("w1r", [128, 4], f32, kind="ExternalInput")
    b1r = nc.dram_tensor("b1r", [128, 4], f32, kind="ExternalInput")
    mo = nc.dram_tensor("mo", [128, 4 * NROWPP], f16, kind="ExternalOutput")

    with tile.TileContext(nc) as tc:
        with tc.tile_pool(name="p", bufs=4) as pool, \
             tc.tile_pool(name="s", bufs=1) as spool:
            seg = spool.tile([128, NROWPP], f32)
            off = 0
            for k, pad in enumerate(pads):
                w = CROWS * pad
                gt = pool.tile([128, CROWS * padmax], f16, tag="g")
                nc.sync.dma_start(out=gt[:, :w], in_=gu.ap()[:, off:off + w])
                nc.vector.tensor_reduce(
                    out=seg[:, k * CROWS:(k + 1) * CROWS],
                    in_=gt[:, :w].rearrange("p (c s) -> p c s", s=pad),
                    axis=mybir.AxisListType.X, op=add)
                off += w
            ut = spool.tile([128, NROWPP], f16, tag="u")
            dvt = spool.tile([128, NROWPP], f32, tag="dv")
            w1t = spool.tile([128, 4], f32, tag="w1")
            b1t = spool.tile([128, 4], f32, tag="b1")
            nc.sync.dma_start(out=ut[:], in_=uo.ap())
            nc.sync.dma_start(out=dvt[:], in_=dv.ap())
            nc.sync.dma_start(out=w1t[:], in_=w1r.ap())
            nc.sync.dma_start(out=b1t[:], in_=b1r.ap())
            uf = spool.tile([128, NROWPP], f32, tag="uf")
            nc.vector.tensor_copy(out=uf[:], in_=ut[:])
            nc.vector.tensor_add(out=seg[:], in0=seg[:], in1=uf[:])
            nc.vector.tensor_tensor(out=seg[:], in0=seg[:], in1=dvt[:],
                                    op=mult)
            mt = spool.tile([128, 4 * NROWPP], f16, tag="m")
            for c in range(4):
                h = pool.tile([128, NROWPP], f32, tag="h")
                nc.scalar.activation(out=h[:], in_=seg[:], func=Relu,
                                     bias=b1t[:, c:c + 1],
                                     scale=w1t[:, c:c + 1])
                nc.vector.tensor_tensor(out=h[:], in0=h[:], in1=dvt[:],
                                        op=mult)
                nc.vector.tensor_copy(
                    out=mt[:, c * NROWPP:(c + 1) * NROWPP], in_=h[:])
            nc.sync.dma_start(out=mo.ap(), in_=mt[:])
    nc.compile()
    return nc


def _build_neff2(pads):
    """Layer 2: S[v,c] = rowsum(M[src,c]); z = dinv*(S+M); out = z@W2+b2."""
    from concourse import bacc, mybir, tile

    nc = bacc.Bacc("TRN2", target_bir_lowering=False, debug=False,
                   num_devices=N_CORES)
    f32, f16 = mybir.dt.float32, mybir.dt.float16
    mult, add = mybir.AluOpType.mult, mybir.AluOpType.add
    Copy = mybir.ActivationFunctionType.Copy
    Ident = mybir.ActivationFunctionType.Identity
    totw = sum(CROWS * p for p in pads)
    padmax = max(pads)

    gm = nc.dram_tensor("gm", [128, 4 * totw], f16, kind="ExternalInput")
    mo = nc.dram_tensor("mo", [128, 4 * NROWPP], f16, kind="ExternalInput")
    dv = nc.dram_tensor("dv", [128, NROWPP], f32, kind="ExternalInput")
    w2r = nc.dram_tensor("w2r", [128, 16], f32, kind="ExternalInput")
    b2r = nc.dram_tensor("b2r", [128, 4], f32, kind="ExternalInput")
    out = nc.dram_tensor("out", [128, NROWPP * 4], f32, kind="ExternalOutput")

    with tile.TileContext(nc) as tc:
        with tc.tile_pool(name="p", bufs=4) as pool, \
             tc.tile_pool(name="q", bufs=2) as psm, \
             tc.tile_pool(name="s", bufs=1) as spool:
            S = spool.tile([128, 4 * NROWPP], f32)
            off = 0
            for k, pad in enumerate(pads):
                w = CROWS * pad
                gt = pool.tile([128, 4 * CROWS * padmax], f16, tag="g")
                nc.sync.dma_start(out=gt[:, :4 * w],
                                  in_=gm.ap()[:, off:off + 4 * w])
                for c in range(4):
                    nc.vector.tensor_reduce(
                        out=S[:, c * NROWPP + k * CROWS:
                              c * NROWPP + (k + 1) * CROWS],
                        in_=gt[:, c * w:(c + 1) * w].rearrange(
                            "p (c s) -> p c s", s=pad),
                        axis=mybir.AxisListType.X, op=add)
                off += 4 * w
            mt = spool.tile([128, 4 * NROWPP], f16, tag="m")
            dvt = spool.tile([128, NROWPP], f32, tag="dv")
            w2t = spool.tile([128, 16], f32, tag="w2")
            b2t = spool.tile([128, 4], f32, tag="b2")
            nc.sync.dma_start(out=mt[:], in_=mo.ap())
            nc.sync.dma_start(out=dvt[:], in_=dv.ap())
            nc.sync.dma_start(out=w2t[:], in_=w2r.ap())
            nc.sync.dma_start(out=b2t[:], in_=b2r.ap())
            mf = spool.tile([128, 4 * NROWPP], f32, tag="mf")
            nc.vector.tensor_copy(out=mf[:], in_=mt[:])
            for c in range(4):
                Sc = S[:, c * NROWPP:(c + 1) * NROWPP]
                nc.vector.tensor_add(
                    out=Sc, in0=Sc, in1=mf[:, c * NROWPP:(c + 1) * NROWPP])
                nc.vector.tensor_tensor(out=Sc, in0=Sc, in1=dvt[:], op=mult)
            ot = spool.tile([128, NROWPP * 4], f32, tag="o")
            o3 = ot[:].rearrange("p (r j) -> p r j", j=4)
            for j in range(4):
                acc = psm.tile([128, NROWPP], f32, tag="acc")
                nc.scalar.activation(out=acc[:], in_=S[:, 0:NROWPP],
                                     func=Copy, scale=w2t[:, j:j + 1])
                for c in range(1, 4):
                    nc.vector.scalar_tensor_tensor(
                        out=acc[:], in0=S[:, c * NROWPP:(c + 1) * NROWPP],
                        scalar=w2t[:, c * 4 + j:c * 4 + j + 1], in1=acc[:],
                        op0=mult, op1=add)
                nc.scalar.activation(out=o3[:, :, j], in_=acc[:], func=Ident,
                                     bias=b2t[:, j:j + 1])
            nc.sync.dma_start(out=out.ap(), in_=ot[:])
    nc.compile()
    return nc


def _get_neffs(pads):
    key = tuple(pads)
    if key not in _NEFF_CACHE:
        _NEFF_CACHE[key] = (_build_neff0(), _build_neff1(pads),
                            _build_neff2(pads))
    return _NEFF_CACHE[key]


def kernel(x, edge_index, W1, b1, W2, b2):
    from concourse import bass_utils

    x = np.asarray(x, dtype=np.float32)
    W1 = np.asarray(W1, dtype=np.float32)
    b1 = np.asarray(b1, dtype=np.float32)
    W2 = np.asarray(W2, dtype=np.float32)
    b2 = np.asarray(b2, dtype=np.float32)
    ei = np.asarray(edge_index)
    assert x.shape == (N_NODES, 1) and ei.shape == (2, N_EDGES)
    xf = np.ascontiguousarray(x.reshape(-1))
    src = ei[0].astype(np.int64)
    dst = ei[1].astype(np.int64)

    # ---- host layout (index work only) ----
    deg = np.bincount(dst, minlength=N_NODES)           # int64, no self loop
    degp1 = (deg + 1).astype(np.uint16)

    # per-core degree sort (desc, stable); rank s -> (p = s%128, r = s//128)
    deg2 = deg.reshape(N_CORES, NPC)
    order = np.argsort(-deg2, axis=1, kind="stable")    # [8, NPC] local ids
    sorted_ids = order + (np.arange(N_CORES)[:, None] * NPC)  # global ids
    srank = np.empty(N_NODES, np.int64)
    srank[sorted_ids.reshape(-1)] = np.tile(np.arange(NPC), N_CORES)

    dsorted = np.take_along_axis(deg2, order, axis=1)   # [8, NPC] descending
    dpad = np.zeros((N_CORES, NROWTOT), np.int64)
    dpad[:, :NPC] = dsorted
    rowmax = dpad.reshape(N_CORES, NROWPP, 128)[:, :, 0].max(axis=0)
    pads = []
    for k in range(NCHUNK):
        m = int(rowmax[k * CROWS])                      # non-increasing
        pads.append(max(8, int(math.ceil(m / 8) * 8)))
    totw = sum(CROWS * p for p in pads)

    # per-row column bases in the two grids
    base1 = np.empty(NROWPP, np.int64)
    base2 = np.empty((NROWPP, 4), np.int64)
    off1 = 0
    off2 = 0
    for k, pad in enumerate(pads):
        r0 = k * CROWS
        rr = np.arange(CROWS, dtype=np.int64)
        base1[r0:r0 + CROWS] = off1 + rr * pad
        for c in range(4):
            base2[r0:r0 + CROWS, c] = off2 + c * CROWS * pad + rr * pad
        off1 += CROWS * pad
        off2 += 4 * CROWS * pad

    # edge -> (core, partition, row, slot)
    key = (dst << 19) | src                             # N_NODES < 2**19
    key.sort(kind="stable")
    sdst = key >> 19
    ssrc = (key & 0x7FFFF).astype(np.int64)
    ptr = np.zeros(N_NODES + 1, np.int64)
    np.cumsum(deg, out=ptr[1:])
    jslot = np.arange(N_EDGES, dtype=np.int64) - ptr[sdst]
    corei = sdst // NPC
    s_e = srank[sdst]
    p_e = s_e & 127
    r_e = s_e >> 7
    flat1 = p_e * totw + base1[r_e] + jslot

    nc0, nc1, nc2 = _get_neffs(pads)

    # ---- NEFF0: per-node u = x*dinv (fp16), dinv (f32) ----
    XO = np.zeros((N_CORES, NROWTOT), np.float32)
    XO[:, :NPC] = xf.reshape(N_CORES, NPC)
    DG = np.ones((N_CORES, NROWTOT), np.uint16)
    DG[:, :NPC] = degp1.reshape(N_CORES, NPC)
    in0 = [{"xo": XO[c].reshape(128, NROWPP),
            "dg": DG[c].reshape(128, NROWPP)} for c in range(N_CORES)]
    res0 = bass_utils.run_bass_kernel_spmd(nc0, in0,
                                           core_ids=list(range(N_CORES)))
    u_full = np.concatenate([
        np.asarray(res0.results[c]["uo"], np.float16).reshape(-1)[:NPC]
        for c in range(N_CORES)])
    dv_full = np.concatenate([
        np.asarray(res0.results[c]["dv"], np.float32).reshape(-1)[:NPC]
        for c in range(N_CORES)])

    # sorted-order self tensors [8, 128, NROWPP]: value at (p, r) = rank r*128+p
    def to_sorted(vals, dtype):
        vs = vals[sorted_ids]                           # [8, NPC] rank order
        arr = np.zeros((N_CORES, NROWTOT), dtype)
        arr[:, :NPC] = vs
        return np.ascontiguousarray(
            arr.reshape(N_CORES, NROWPP, 128).transpose(0, 2, 1))

    UO = to_sorted(u_full, np.float16)
    DVO = to_sorted(dv_full, np.float32)

    # ---- NEFF1: layer-1 grid of u[src] ----
    GU = np.zeros((N_CORES, 128 * totw), np.uint16)
    GU[corei, flat1] = u_full.view(np.uint16)[ssrc]
    w1r = np.tile(W1.reshape(1, 4), (128, 1)).astype(np.float32)
    b1r = np.tile(b1.reshape(1, 4), (128, 1)).astype(np.float32)
    in1 = [{"gu": GU[c].view(np.float16).reshape(128, totw),
            "uo": UO[c], "dv": DVO[c], "w1r": w1r, "b1r": b1r}
           for c in range(N_CORES)]
    res1 = bass_utils.run_bass_kernel_spmd(nc1, in1,
                                           core_ids=list(range(N_CORES)))
    m_raw = [np.ascontiguousarray(
        np.asarray(res1.results[c]["mo"], np.float16).reshape(128, 4 * NROWPP))
        for c in range(N_CORES)]

    # M planes per node (global), channel-planar
    M_full = np.empty((4, N_NODES), np.float16)
    for c in range(N_CORES):
        for ch in range(4):
            plane = m_raw[c][:, ch * NROWPP:(ch + 1) * NROWPP]
            M_full[ch, sorted_ids[c]] = plane.T.reshape(-1)[:NPC]

    # ---- NEFF2: 4-channel grid of M[src] ----
    GM = np.zeros((N_CORES, 128 * 4 * totw), np.uint16)
    pbase = p_e * (4 * totw) + jslot
    for ch in range(4):
        GM[corei, pbase + base2[r_e, ch]] = M_full[ch].view(np.uint16)[ssrc]
    w2r = np.tile(W2.reshape(1, 16), (128, 1)).astype(np.float32)
    b2r = np.tile(b2.reshape(1, 4), (128, 1)).astype(np.float32)
    in2 = [{"gm": GM[c].view(np.float16).reshape(128, 4 * totw),
            "mo": m_raw[c], "dv": DVO[c], "w2r": w2r, "b2r": b2r}
           for c in range(N_CORES)]
    res2 = bass_utils.run_bass_kernel_spmd(nc2, in2,
                                           core_ids=list(range(N_CORES)))

    out = np.empty((N_NODES, 4), np.float32)
    for c in range(N_CORES):
        O = np.asarray(res2.results[c]["out"], np.float32).reshape(
            128, NROWPP, 4)
        out[sorted_ids[c]] = O.transpose(1, 0, 2).reshape(NROWTOT, 4)[:NPC]
    return np.ascontiguousarray(out)


# revision 3
# speedup vs baseline: 4.8793x; 1.0746x over previous
"""GCN (2-layer, PyG GCNConv semantics) on 8 Trainium2 NeuronCores.

Strategy
--------
All device work is DENSE row-sums over host-built, dst-sorted edge grids
(per-edge gather/scatter on TRN2 is far too slow). Versus the naive
grid approach, the kernel stays near the HBM roofline by:

1. No per-slot normalization math. A tiny NEFF0 computes per-NODE
   u = dinv*x and dinv (dinv = 1/sqrt(deg+1)); the host gathers the
   already-normalized per-node fp16 values into the edge grids, so the
   big NEFFs only do segment sums.
2. fp16 grids: halves HBM traffic and doubles DVE throughput.
   (fp16 values are produced ON DEVICE; the host only moves bytes.)
3. Degree-sorted rows with per-chunk padding: nodes are sorted by
   degree (desc) inside each core, rows take 128 consecutive ranks, and
   each chunk of rows is padded to its own max degree -> padding
   inflation ~1.19x instead of maxdeg/meandeg ~2.5x.
4. Segment sums via in-place pairwise fold adds on a SLOT-MAJOR grid
   layout ([slot, chan, row] per chunk): contiguous fp16 tensor_tensor
   adds run in the DVE 2x packed mode (4 inputs/cycle), ~3x faster
   than TENSOR_REDUCE over short padded rows (1/cycle + ~20cy/row
   restart penalty).

Math (A = D^-1/2 (Adj+I) D^-1/2, deg counts in-edges at dst +1):
  y1[v]   = dinv[v]*(sum_{e->v} u[src] + u[v]),  u = dinv*x
  M[v,c]  = dinv[v]*relu(W1[0,c]*y1[v] + b1[c])
  z[v,c]  = dinv[v]*(sum_{e->v} M[src,c] + M[v,c])
  out     = z @ W2 + b2

NEFF0: per-node u, dinv.  NEFF1: layer-1 segment sums + M.  NEFF2:
4-channel segment sums + W2 combine. Host work between launches is
pure index work (sort/gather/scatter/pad of device-produced bytes).
"""
import math
import sys

sys.path.insert(0, "/opt/trn_rl_repo")

import numpy as np

N_NODES = 500_000
N_EDGES = 16_000_000
N_CORES = 8
NPC = N_NODES // N_CORES        # 62500 nodes per core
NROWPP = 496                    # rows per partition (128*496 = 63488 >= NPC)
NROWTOT = 128 * NROWPP
NCHUNK = 16
CROWS = NROWPP // NCHUNK        # 31 rows per partition per chunk

_NEFF_CACHE: dict = {}


def _tree_sum16(nc, pool, f16, leaves, out_ap):
    """Pairwise-add fp16 leaf APs [128, G]; final add writes f32 out_ap."""
    G = leaves[0].shape[-1]
    while len(leaves) > 2:
        nxt = []
        for i in range(0, len(leaves) - 1, 2):
            t = pool.tile([128, G], f16, tag="tsum")
            nc.vector.tensor_add(out=t[:], in0=leaves[i], in1=leaves[i + 1])
            nxt.append(t[:])
        if len(leaves) % 2:
            nxt.append(leaves[-1])
        leaves = nxt
    if len(leaves) == 2:
        nc.vector.tensor_add(out=out_ap, in0=leaves[0], in1=leaves[1])
    else:
        nc.vector.tensor_copy(out=out_ap, in_=leaves[0])


def _fold_chunk(nc, pool, f16, gt, pad, unit, out_ap):
    """Segment-sum a slot-major chunk tile gt [128, pad*unit] (slot index
    outer, unit = chans*rows inner) into f32 out_ap [128, unit] via
    in-place halving adds while even, then a small tree."""
    s = pad
    while s % 2 == 0 and s > 2:
        h = (s // 2) * unit
        nc.vector.tensor_add(out=gt[:, :h], in0=gt[:, :h], in1=gt[:, h:2 * h])
        s //= 2
    if s == 2:
        nc.vector.tensor_add(out=out_ap, in0=gt[:, :unit],
                             in1=gt[:, unit:2 * unit])
    elif s == 1:
        nc.vector.tensor_copy(out=out_ap, in_=gt[:, :unit])
    else:
        leaves = [gt[:, i * unit:(i + 1) * unit] for i in range(s)]
        _tree_sum16(nc, pool, f16, leaves, out_ap)


def _build_neff0():
    """Per-node: dinv = 1/sqrt(deg+1); u = x*dinv (fp16 out)."""
    from concourse import bacc, mybir, tile

    nc = bacc.Bacc("TRN2", target_bir_lowering=False, debug=False,
                   num_devices=N_CORES)
    f32, f16, u16 = mybir.dt.float32, mybir.dt.float16, mybir.dt.uint16
    xo = nc.dram_tensor("xo", [128, NROWPP], f32, kind="ExternalInput")
    dg = nc.dram_tensor("dg", [128, NROWPP], u16, kind="ExternalInput")
    uo = nc.dram_tensor("uo", [128, NROWPP], f16, kind="ExternalOutput")
    dv = nc.dram_tensor("dv", [128, NROWPP], f32, kind="ExternalOutput")

    with tile.TileContext(nc) as tc:
        with tc.tile_pool(name="p", bufs=2) as pool:
            half = NROWPP // 2
            for i in range(2):
                sl = slice(i * half, (i + 1) * half)
                sh = [128, half]
                xt = pool.tile(sh, f32, tag="x")
                dt_ = pool.tile(sh, u16, tag="d")
                nc.sync.dma_start(out=xt[:], in_=xo.ap()[:, sl])
                nc.sync.dma_start(out=dt_[:], in_=dg.ap()[:, sl])
                df = pool.tile(sh, f32, tag="df")
                nc.vector.tensor_copy(out=df[:], in_=dt_[:])
                rc = pool.tile(sh, f32, tag="rc")
                nc.vector.reciprocal_approx_fast(out=rc[:], in_=df[:])
                dvt = pool.tile(sh, f32, tag="dv")
                nc.scalar.sqrt(out=dvt[:], in_=rc[:])
                ut = pool.tile(sh, f16, tag="u")
                nc.vector.tensor_tensor(out=ut[:], in0=xt[:], in1=dvt[:],
                                        op=mybir.AluOpType.mult)
                nc.sync.dma_start(out=uo.ap()[:, sl], in_=ut[:])
                nc.sync.dma_start(out=dv.ap()[:, sl], in_=dvt[:])
    nc.compile()
    return nc


def _build_neff1(pads):
    """Layer 1: seg[v] = sum(u[src]); y1 = dinv*(seg+u);
    M[v,c] = dinv*relu(W1c*y1+b1c) -> fp16 planes [128, 4*NROWPP]."""
    from concourse import bacc, mybir, tile

    nc = bacc.Bacc("TRN2", target_bir_lowering=False, debug=False,
                   num_devices=N_CORES)
    f32, f16 = mybir.dt.float32, mybir.dt.float16
    mult = mybir.AluOpType.mult
    Relu = mybir.ActivationFunctionType.Relu
    totw = sum(CROWS * p for p in pads)
    padmax = max(pads)

    gu = nc.dram_tensor("gu", [128, totw], f16, kind="ExternalInput")
    uo = nc.dram_tensor("uo", [128, NROWPP], f16, kind="ExternalInput")
    dv = nc.dram_tensor("dv", [128, NROWPP], f32, kind="ExternalInput")
    w1r = nc.dram_tensor("w1r", [128, 4], f32, kind="ExternalInput")
    b1r = nc.dram_tensor("b1r", [128, 4], f32, kind="ExternalInput")
    mo = nc.dram_tensor("mo", [128, 4 * NROWPP], f16, kind="ExternalOutput")

    with tile.TileContext(nc) as tc:
        with tc.tile_pool(name="p", bufs=4) as pool, \
             tc.tile_pool(name="t", bufs=4) as tpool, \
             tc.tile_pool(name="s", bufs=1) as spool:
            seg = spool.tile([128, NROWPP], f32)
            off = 0
            for k, pad in enumerate(pads):
                w = CROWS * pad
                gt = pool.tile([128, CROWS * padmax], f16, tag="g")
                nc.sync.dma_start(out=gt[:, :w], in_=gu.ap()[:, off:off + w])
                _fold_chunk(nc, tpool, f16, gt, pad, CROWS,
                            seg[:, k * CROWS:(k + 1) * CROWS])
                off += w
            ut = spool.tile([128, NROWPP], f16, tag="u")
            dvt = spool.tile([128, NROWPP], f32, tag="dv")
            w1t = spool.tile([128, 4], f32, tag="w1")
            b1t = spool.tile([128, 4], f32, tag="b1")
            nc.sync.dma_start(out=ut[:], in_=uo.ap())
            nc.sync.dma_start(out=dvt[:], in_=dv.ap())
            nc.sync.dma_start(out=w1t[:], in_=w1r.ap())
            nc.sync.dma_start(out=b1t[:], in_=b1r.ap())
            uf = spool.tile([128, NROWPP], f32, tag="uf")
            nc.vector.tensor_copy(out=uf[:], in_=ut[:])
            nc.vector.tensor_add(out=seg[:], in0=seg[:], in1=uf[:])
            nc.vector.tensor_tensor(out=seg[:], in0=seg[:], in1=dvt[:],
                                    op=mult)
            for c in range(4):
                h = tpool.tile([128, NROWPP], f32, tag="h")
                nc.scalar.activation(out=h[:], in_=seg[:], func=Relu,
                                     bias=b1t[:, c:c + 1],
                                     scale=w1t[:, c:c + 1])
                m16 = tpool.tile([128, NROWPP], f16, tag="m16")
                nc.vector.tensor_tensor(out=m16[:], in0=h[:], in1=dvt[:],
                                        op=mult)
                nc.sync.dma_start(
                    out=mo.ap()[:, c * NROWPP:(c + 1) * NROWPP], in_=m16[:])
    nc.compile()
    return nc


def _build_neff2(pads):
    """Layer 2: S[v,c] = sum(M[src,c]); z = dinv*(S+M); out = z@W2+b2
    (out planar [128, 4*NROWPP], channel-major)."""
    from concourse import bacc, mybir, tile

    nc = bacc.Bacc("TRN2", target_bir_lowering=False, debug=False,
                   num_devices=N_CORES)
    f32, f16 = mybir.dt.float32, mybir.dt.float16
    mult, add = mybir.AluOpType.mult, mybir.AluOpType.add
    Copy = mybir.ActivationFunctionType.Copy
    Ident = mybir.ActivationFunctionType.Identity
    totw = sum(CROWS * p for p in pads)
    padmax = max(pads)
    UNIT = 4 * CROWS

    gm = nc.dram_tensor("gm", [128, 4 * totw], f16, kind="ExternalInput")
    mo = nc.dram_tensor("mo", [128, 4 * NROWPP], f16, kind="ExternalInput")
    dv = nc.dram_tensor("dv", [128, NROWPP], f32, kind="ExternalInput")
    w2r = nc.dram_tensor("w2r", [128, 16], f32, kind="ExternalInput")
    b2r = nc.dram_tensor("b2r", [128, 4], f32, kind="ExternalInput")
    out = nc.dram_tensor("out", [128, 4 * NROWPP], f32, kind="ExternalOutput")

    with tile.TileContext(nc) as tc:
        with tc.tile_pool(name="p", bufs=4) as pool, \
             tc.tile_pool(name="t", bufs=4) as tpool, \
             tc.tile_pool(name="q", bufs=2) as psm, \
             tc.tile_pool(name="s", bufs=1) as spool:
            # interleaved per-chunk sums: Sint[p, (k c r)]
            Sint = spool.tile([128, NCHUNK * UNIT], f32, tag="si")
            off = 0
            for k, pad in enumerate(pads):
                w = UNIT * pad
                gt = pool.tile([128, UNIT * padmax], f16, tag="g")
                nc.sync.dma_start(out=gt[:, :w], in_=gm.ap()[:, off:off + w])
                _fold_chunk(nc, tpool, f16, gt, pad, UNIT,
                            Sint[:, k * UNIT:(k + 1) * UNIT])
                off += w
            mt = spool.tile([128, 4 * NROWPP], f16, tag="m")
            dvt = spool.tile([128, NROWPP], f32, tag="dv")
            w2t = spool.tile([128, 16], f32, tag="w2")
            b2t = spool.tile([128, 4], f32, tag="b2")
            nc.sync.dma_start(out=mt[:], in_=mo.ap())
            nc.sync.dma_start(out=dvt[:], in_=dv.ap())
            nc.sync.dma_start(out=w2t[:], in_=w2r.ap())
            nc.sync.dma_start(out=b2t[:], in_=b2r.ap())
            # reorder to channel-planar S[p, (c k r)]
            S = spool.tile([128, 4 * NROWPP], f32, tag="sp")
            si4 = Sint[:].rearrange("p (k c r) -> p k c r", c=4, r=CROWS)
            for c in range(4):
                nc.vector.tensor_copy(
                    out=S[:, c * NROWPP:(c + 1) * NROWPP].rearrange(
                        "p (k r) -> p k r", r=CROWS),
                    in_=si4[:, :, c, :])
            mf = spool.tile([128, 4 * NROWPP], f32, tag="mf")
            nc.vector.tensor_copy(out=mf[:], in_=mt[:])
            nc.vector.tensor_add(out=S[:], in0=S[:], in1=mf[:])
            for c in range(4):
                Sc = S[:, c * NROWPP:(c + 1) * NROWPP]
                nc.vector.tensor_tensor(out=Sc, in0=Sc, in1=dvt[:], op=mult)
            for j in range(4):
                acc = psm.tile([128, NROWPP], f32, tag="acc")
                nc.scalar.activation(out=acc[:], in_=S[:, 0:NROWPP],
                                     func=Copy, scale=w2t[:, j:j + 1])
                for c in range(1, 4):
                    nc.vector.scalar_tensor_tensor(
                        out=acc[:], in0=S[:, c * NROWPP:(c + 1) * NROWPP],
                        scalar=w2t[:, c * 4 + j:c * 4 + j + 1], in1=acc[:],
                        op0=mult, op1=add)
                oj = psm.tile([128, NROWPP], f32, tag="oj")
                nc.scalar.activation(out=oj[:], in_=acc[:], func=Ident,
                                     bias=b2t[:, j:j + 1])
                nc.sync.dma_start(
                    out=out.ap()[:, j * NROWPP:(j + 1) * NROWPP], in_=oj[:])
    nc.compile()
    return nc


def _get_neffs(pads):
    key = tuple(pads)
    if key not in _NEFF_CACHE:
        _NEFF_CACHE[key] = (_build_neff0(), _build_neff1(pads),
                            _build_neff2(pads))
    return _NEFF_CACHE[key]


def kernel(x, edge_index, W1, b1, W2, b2):
    from concourse import bass_utils

    x = np.asarray(x, dtype=np.float32)
    W1 = np.asarray(W1, dtype=np.float32)
    b1 = np.asarray(b1, dtype=np.float32)
    W2 = np.asarray(W2, dtype=np.float32)
    b2 = np.asarray(b2, dtype=np.float32)
    ei = np.asarray(edge_index)
    assert x.shape == (N_NODES, 1) and ei.shape == (2, N_EDGES)
    xf = np.ascontiguousarray(x.reshape(-1))
    src = ei[0].astype(np.int64)
    dst = ei[1].astype(np.int64)

    # ---- host layout (index work only) ----
    deg = np.bincount(dst, minlength=N_NODES)           # int64, no self loop
    degp1 = (deg + 1).astype(np.uint16)

    # per-core degree sort (desc, stable); rank s -> (p = s%128, r = s//128)
    deg2 = deg.reshape(N_CORES, NPC)
    order = np.argsort(-deg2, axis=1, kind="stable")    # [8, NPC] local ids
    sorted_ids = order + (np.arange(N_CORES)[:, None] * NPC)  # global ids
    srank = np.empty(N_NODES, np.int64)
    srank[sorted_ids.reshape(-1)] = np.tile(np.arange(NPC), N_CORES)

    dsorted = np.take_along_axis(deg2, order, axis=1)   # [8, NPC] descending
    dpad = np.zeros((N_CORES, NROWTOT), np.int64)
    dpad[:, :NPC] = dsorted
    rowmax = dpad.reshape(N_CORES, NROWPP, 128)[:, :, 0].max(axis=0)
    pads = []
    for k in range(NCHUNK):
        m = int(rowmax[k * CROWS])                      # non-increasing
        pads.append(max(8, int(math.ceil(m / 8) * 8)))
    totw = sum(CROWS * p for p in pads)

    # per-row offsets: chunk base + (r - r0); slot j adds j*CROWS (grid 1)
    # or j*4*CROWS (grid 2, with + c*CROWS for channel c)
    roff1 = np.empty(NROWPP, np.int64)
    roff2 = np.empty(NROWPP, np.int64)
    off1 = 0
    off2 = 0
    for k, pad in enumerate(pads):
        r0 = k * CROWS
        rr = np.arange(CROWS, dtype=np.int64)
        roff1[r0:r0 + CROWS] = off1 + rr
        roff2[r0:r0 + CROWS] = off2 + rr
        off1 += CROWS * pad
        off2 += 4 * CROWS * pad

    # edge -> (core, partition, row, slot)
    key = (dst << 19) | src                             # N_NODES < 2**19
    key.sort(kind="stable")
    sdst = key >> 19
    ssrc = (key & 0x7FFFF).astype(np.int64)
    ptr = np.zeros(N_NODES + 1, np.int64)
    np.cumsum(deg, out=ptr[1:])
    jslot = np.arange(N_EDGES, dtype=np.int64) - ptr[sdst]
    corei = sdst // NPC
    s_e = srank[sdst]
    p_e = s_e & 127
    r_e = s_e >> 7
    flat1 = p_e * totw + roff1[r_e] + jslot * CROWS

    nc0, nc1, nc2 = _get_neffs(pads)

    # ---- NEFF0: per-node u = x*dinv (fp16), dinv (f32) ----
    XO = np.zeros((N_CORES, NROWTOT), np.float32)
    XO[:, :NPC] = xf.reshape(N_CORES, NPC)
    DG = np.ones((N_CORES, NROWTOT), np.uint16)
    DG[:, :NPC] = degp1.reshape(N_CORES, NPC)
    in0 = [{"xo": XO[c].reshape(128, NROWPP),
            "dg": DG[c].reshape(128, NROWPP)} for c in range(N_CORES)]
    res0 = bass_utils.run_bass_kernel_spmd(nc0, in0,
                                           core_ids=list(range(N_CORES)))
    u_full = np.concatenate([
        np.asarray(res0.results[c]["uo"], np.float16).reshape(-1)[:NPC]
        for c in range(N_CORES)])
    dv_full = np.concatenate([
        np.asarray(res0.results[c]["dv"], np.float32).reshape(-1)[:NPC]
        for c in range(N_CORES)])

    # sorted-order self tensors [8, 128, NROWPP]: value at (p, r) = rank r*128+p
    def to_sorted(vals, dtype):
        vs = vals[sorted_ids]                           # [8, NPC] rank order
        arr = np.zeros((N_CORES, NROWTOT), dtype)
        arr[:, :NPC] = vs
        return np.ascontiguousarray(
            arr.reshape(N_CORES, NROWPP, 128).transpose(0, 2, 1))

    UO = to_sorted(u_full, np.float16)
    DVO = to_sorted(dv_full, np.float32)

    # ---- NEFF1: layer-1 grid of u[src], slot-major per chunk ----
    GU = np.zeros((N_CORES, 128 * totw), np.uint16)
    GU[corei, flat1] = u_full.view(np.uint16)[ssrc]
    w1r = np.tile(W1.reshape(1, 4), (128, 1)).astype(np.float32)
    b1r = np.tile(b1.reshape(1, 4), (128, 1)).astype(np.float32)
    in1 = [{"gu": GU[c].view(np.float16).reshape(128, totw),
            "uo": UO[c], "dv": DVO[c], "w1r": w1r, "b1r": b1r}
           for c in range(N_CORES)]
    res1 = bass_utils.run_bass_kernel_spmd(nc1, in1,
                                           core_ids=list(range(N_CORES)))
    m_raw = [np.ascontiguousarray(
        np.asarray(res1.results[c]["mo"], np.float16).reshape(128, 4 * NROWPP))
        for c in range(N_CORES)]

    # M planes per node (global), channel-planar
    M_full = np.empty((4, N_NODES), np.float16)
    for c in range(N_CORES):
        for ch in range(4):
            plane = m_raw[c][:, ch * NROWPP:(ch + 1) * NROWPP]
            M_full[ch, sorted_ids[c]] = plane.T.reshape(-1)[:NPC]

    # ---- NEFF2: 4-channel grid of M[src], slot-major [s, c, r] per chunk ----
    GM = np.zeros((N_CORES, 128 * 4 * totw), np.uint16)
    pbase = p_e * (4 * totw) + roff2[r_e] + jslot * (4 * CROWS)
    for ch in range(4):
        GM[corei, pbase + ch * CROWS] = M_full[ch].view(np.uint16)[ssrc]
    w2r = np.tile(W2.reshape(1, 16), (128, 1)).astype(np.float32)
    b2r = np.tile(b2.reshape(1, 4), (128, 1)).astype(np.float32)
    in2 = [{"gm": GM[c].view(np.float16).reshape(128, 4 * totw),
            "mo": m_raw[c], "dv": DVO[c], "w2r": w2r, "b2r": b2r}
           for c in range(N_CORES)]
    res2 = bass_utils.run_bass_kernel_spmd(nc2, in2,
                                           core_ids=list(range(N_CORES)))

    out = np.empty((N_NODES, 4), np.float32)
    for c in range(N_CORES):
        O = np.asarray(res2.results[c]["out"], np.float32).reshape(
            128, 4, NROWPP)
        # O[p, j, r] -> rank s = r*128+p
        out[sorted_ids[c]] = O.transpose(2, 0, 1).reshape(NROWTOT, 4)[:NPC]
    return np.ascontiguousarray(out)
